# revision 19
# baseline (speedup 1.0000x reference)
"""GQA (16 q-heads / 4 KV groups, S=4096, D=1024, causal) on 8 TRN2 NeuronCores.

Sharding: tensor-parallel over query heads — 2 q-heads + their KV group per
core. wq/wk/wv column-sharded, wo row-sharded; the 8 partial outputs are
summed on the host (no device collectives needed).

Per-core program (all matmuls bf16, f32 PSUM accumulation):
  qT   = (wq_c @ x^T)            [128, 4096]   (2 heads x 64 dims, transposed)
  kvT  = (wkv_c @ x^T)           [128, 4096]   (rows 0-63 kT, 64-127 vT)
  v    = transpose(vT) + ones col               (DMA transpose, [128,65] tiles)
  per (q-chunk qc of 512, head h):
    sT[k,q] = kT_kt^T . qT_h     (K=64 matmuls, PSUM [128,3,512] groups)
    pT = exp(sT/8)               (one ACT inst per 3-tile group)
    causal mask on diagonal strips (DVE memset + tri-mask mul)
    ctxT[65,512] += v_aug_kt^T . pT   (row 64 = softmax denominators)
    denominators -> DRAM -> [128,4] -> reciprocal -> DRAM -> broadcast
    ctxT normalized in-place (DVE), then out rows = ctxT_chunk^T @ woT
Softmax uses no max-subtraction: s/8 ~ N(0,1), max ~ 10 -> exp safe in f32.
"""

import numpy as np
import ml_dtypes

BF16 = ml_dtypes.bfloat16

S = 4096
DIN = 1024
DIM = 1024
NH, NKV, HD = 16, 4, 64
NCORES = 8
QC = 512          # q chunk width
NQC = S // QC     # 8
NKT = S // 128    # 32 k tiles
GROUP = 3         # k-tiles per ACT exp instruction (3 PSUM banks)

_CACHE = {}


def _build_nc(debug=False):
    import concourse.bass as bass
    import concourse.mybir as mybir
    import concourse.tile as tile
    from concourse import bacc
    from concourse.tile_rust import add_dep_helper
    from contextlib import ExitStack

    fp32 = mybir.dt.float32
    bf16 = mybir.dt.bfloat16
    Exp = mybir.ActivationFunctionType.Exp

    nc = bacc.Bacc()
    xT_d = nc.dram_tensor("xT", [DIN, S], bf16, kind="ExternalInput")
    wqT_d = nc.dram_tensor("wqT", [DIN, 128], bf16, kind="ExternalInput")
    wkvT_d = nc.dram_tensor("wkvT", [DIN, 128], bf16, kind="ExternalInput")
    woT_d = nc.dram_tensor("woT", [128, DIM], bf16, kind="ExternalInput")
    mask_d = nc.dram_tensor("trimask", [128, 128], bf16, kind="ExternalInput")
    out_d = nc.dram_tensor("out", [S, DIM], fp32, kind="ExternalOutput")
    skind = {"kind": "ExternalOutput"} if debug else {}
    sums_d = nc.dram_tensor("sums_scratch", [2, S], fp32, **skind)
    rec_d = nc.dram_tensor("recips_scratch", [2, S], bf16, **skind)
    if debug:
        dbg_qT = nc.dram_tensor("dbg_qT", [128, S], bf16, kind="ExternalOutput")
        dbg_kvT = nc.dram_tensor("dbg_kvT", [128, S], bf16, kind="ExternalOutput")
        dbg_vaug = nc.dram_tensor("dbg_vaug", [128, NKT, 128], bf16, kind="ExternalOutput")
        dbg_ctxT = nc.dram_tensor("dbg_ctxT", [64, 2, S], bf16, kind="ExternalOutput")

    with ExitStack() as ctx:
        tc = ctx.enter_context(tile.TileContext(nc))
        singles = ctx.enter_context(tc.tile_pool(name="singles", bufs=1))
        pt_pool = ctx.enter_context(tc.tile_pool(name="pt", bufs=4))
        small = ctx.enter_context(tc.tile_pool(name="small", bufs=3))
        ostage = ctx.enter_context(tc.tile_pool(name="ostage", bufs=3))
        psum = ctx.enter_context(tc.tile_pool(name="psum", bufs=2, space="PSUM"))

        # ---- constant / persistent SBUF tensors ----
        xT_sb = singles.tile([128, 8, S], bf16, tag="xT")
        wqT_sb = singles.tile([128, 8, 128], bf16, tag="wqT")
        wkvT_sb = singles.tile([128, 8, 128], bf16, tag="wkvT")
        woT_sb = singles.tile([64, 2, DIM], bf16, tag="woT")
        mask_sb = singles.tile([128, 128], bf16, tag="mask")
        qT_sb = singles.tile([128, S], bf16, tag="qT")
        qT1_sb = singles.tile([64, S], bf16, tag="qT1")
        kvT_sb = singles.tile([128, S], bf16, tag="kvT")
        vaug_sb = singles.tile([128, NKT, 128], bf16, tag="vaug")
        ctxT_sb = singles.tile([64, 2, S], bf16, tag="ctxT")

        for c in range(8):
            nc.sync.dma_start(
                out=xT_sb[:, c, :],
                in_=xT_d[:].rearrange("(c p) s -> c p s", p=128)[c],
            )
        nc.sync.dma_start(
            out=wqT_sb, in_=wqT_d[:].rearrange("(c p) m -> p c m", p=128)
        )
        nc.sync.dma_start(
            out=wkvT_sb, in_=wkvT_d[:].rearrange("(c p) m -> p c m", p=128)
        )
        nc.sync.dma_start(
            out=woT_sb, in_=woT_d[:].rearrange("(h p) e -> p h e", p=64)
        )
        nc.sync.dma_start(out=mask_sb, in_=mask_d[:])

        # ---- projections: qT and kvT ----
        for dst, w_sb in ((qT_sb, wqT_sb), (kvT_sb, wkvT_sb)):
            for n in range(NQC):
                ps = psum.tile([128, GROUP, QC], fp32, tag="ps_s")
                for c in range(8):
                    nc.tensor.matmul(
                        ps[:, 0, :],
                        w_sb[:, c, :],
                        xT_sb[:, c, n * QC:(n + 1) * QC],
                        start=(c == 0),
                        stop=(c == 7),
                    )
                nc.vector.tensor_copy(dst[:, n * QC:(n + 1) * QC], ps[:, 0, :])

        # head-1 q rows shifted to base partition 0 (matmul operand rule)
        for n in range(NQC):
            nc.sync.dma_start(
                out=qT1_sb[:, n * QC:(n + 1) * QC],
                in_=qT_sb[64:128, n * QC:(n + 1) * QC],
            )

        # ---- v (normal layout) + ones column ----
        nc.vector.memset(vaug_sb[:, :, 64:66], 1.0)
        for kt in range(NKT):
            nc.sync.dma_start_transpose(
                out=vaug_sb[:, kt, 0:64],
                in_=kvT_sb[64:128, kt * 128:(kt + 1) * 128],
            )

        # ---- attention + output, pipelined over q-chunks ----
        for qc in range(NQC):
            nkt = min(NKT, 4 * qc + 4)
            rec_writes = []
            for h in range(2):
                ctx_ps = psum.tile([65, QC], fp32, tag="ps_ctx")
                if h == 0:
                    qs = qT_sb[0:64, qc * QC:(qc + 1) * QC]
                else:
                    qs = qT1_sb[:, qc * QC:(qc + 1) * QC]
                for g0 in range(0, nkt, GROUP):
                    gn = min(GROUP, nkt - g0)
                    ps_s = psum.tile([128, GROUP, QC], fp32, tag="ps_s")
                    pt = pt_pool.tile([128, GROUP, QC], bf16, tag="pt")
                    for i in range(gn):
                        kt = g0 + i
                        nc.tensor.matmul(
                            ps_s[:, i, :],
                            kvT_sb[0:64, kt * 128:(kt + 1) * 128],
                            qs,
                            start=True,
                            stop=True,
                        )
                    nc.scalar.activation(
                        pt[:, 0:gn, :], ps_s[:, 0:gn, :], Exp, scale=0.125
                    )
                    for i in range(gn):
                        kt = g0 + i
                        r = kt - 4 * qc
                        if r >= 0:  # strip intersects the causal diagonal
                            if r >= 1:
                                nc.vector.memset(pt[:, i, 0:128 * r], 0.0)
                            nc.vector.tensor_mul(
                                pt[:, i, 128 * r:128 * (r + 1)],
                                pt[:, i, 128 * r:128 * (r + 1)],
                                mask_sb,
                            )
                    for i in range(gn):
                        kt = g0 + i
                        nc.tensor.matmul(
                            ctx_ps,
                            vaug_sb[:, kt, 0:65],
                            pt[:, i, :],
                            start=(kt == 0),
                            stop=(kt == nkt - 1),
                        )

                # ctx (raw) -> SBUF; denominators -> DRAM -> recip -> DRAM
                nc.vector.tensor_copy(
                    ctxT_sb[:, h, qc * QC:(qc + 1) * QC],
                    ctx_ps[0:64, :],
                )
                srow = small.tile([65, QC], fp32, tag="srow")
                nc.vector.tensor_copy(srow[64:65, :], ctx_ps[64:65, :])
                w1 = nc.sync.dma_start(
                    out=sums_d[h:h + 1, qc * QC:(qc + 1) * QC],
                    in_=srow[64:65, :],
                )
                cp = small.tile([128, 4], fp32, tag="cp")
                r1 = nc.sync.dma_start(
                    out=cp,
                    in_=sums_d[h, qc * QC:(qc + 1) * QC].rearrange(
                        "(c p) -> p c", p=128
                    ),
                )
                add_dep_helper(r1.ins, w1.ins, reason="sums dram RAW")
                rec = small.tile([128, 4], fp32, tag="rec")
                nc.vector.reciprocal(rec, cp)
                recb = small.tile([128, 4], bf16, tag="recb")
                nc.vector.tensor_copy(recb, rec)
                w2 = nc.sync.dma_start(
                    out=rec_d[h, qc * QC:(qc + 1) * QC].rearrange(
                        "(c p) -> p c", p=128
                    ),
                    in_=recb,
                )
                rb = small.tile([128, QC], bf16, tag="rb")
                src = rec_d[h, qc * QC:(qc + 1) * QC]
                import concourse.bass as _b
                r2 = nc.sync.dma_start(
                    out=rb,
                    in_=_b.AP(tensor=src.tensor, offset=src.offset,
                              ap=[[0, 128]] + list(src.ap)),
                )
                add_dep_helper(r2.ins, w2.ins, reason="recips dram RAW")
                rec_writes.append((rb, None))
                # normalize ctxT in place
                nc.vector.tensor_mul(
                    ctxT_sb[:, h, qc * QC:(qc + 1) * QC],
                    ctxT_sb[:, h, qc * QC:(qc + 1) * QC],
                    rb[0:64, :],
                )

            # ---- output projection for this q-chunk's 4 row blocks ----
            for j, rc in enumerate(range(4 * qc, 4 * qc + 4)):
                ps_o = psum.tile([128, GROUP, QC], fp32, tag="ps_s")
                for e in range(2):
                    for h in range(2):
                        nc.tensor.matmul(
                            ps_o[:, e, :],
                            ctxT_sb[:, h, rc * 128:(rc + 1) * 128],
                            woT_sb[:, h, e * 512:(e + 1) * 512],
                            start=(h == 0),
                            stop=(h == 1),
                        )
                ot = ostage.tile([128, DIM], fp32, tag="ot")
                if j % 2 == 0:
                    nc.vector.tensor_copy(ot[:, 0:512], ps_o[:, 0, :])
                    nc.vector.tensor_copy(ot[:, 512:1024], ps_o[:, 1, :])
                else:
                    nc.scalar.copy(ot[:, 0:512], ps_o[:, 0, :])
                    nc.scalar.copy(ot[:, 512:1024], ps_o[:, 1, :])
                nc.sync.dma_start(
                    out=out_d[rc * 128:(rc + 1) * 128, :], in_=ot
                )

        if debug:
            nc.sync.dma_start(out=dbg_qT[:], in_=qT_sb)
            nc.sync.dma_start(out=dbg_kvT[:], in_=kvT_sb)
            nc.sync.dma_start(out=dbg_vaug[:], in_=vaug_sb)
            nc.sync.dma_start(out=dbg_ctxT[:], in_=ctxT_sb)

    nc.compile()
    return nc


def _get_nc():
    if "nc" not in _CACHE:
        _CACHE["nc"] = _build_nc()
    return _CACHE["nc"]


def _prep_inputs(x, wq, wk, wv, wo):
    GS = NH // NKV
    x2 = np.asarray(x, np.float32).reshape(S, DIN)
    xT = np.ascontiguousarray(x2.T).astype(BF16)
    tri = (np.arange(128)[None, :] >= np.arange(128)[:, None]).astype(BF16)
    in_maps = []
    for c in range(NCORES):
        h0 = 2 * c
        g = h0 // GS
        wq_c = np.asarray(wq, np.float32)[h0 * HD:(h0 + 2) * HD, :]
        wkv_c = np.concatenate(
            [
                np.asarray(wk, np.float32)[g * HD:(g + 1) * HD, :],
                np.asarray(wv, np.float32)[g * HD:(g + 1) * HD, :],
            ],
            axis=0,
        )
        woT_c = np.asarray(wo, np.float32)[:, h0 * HD:(h0 + 2) * HD].T
        in_maps.append(
            {
                "xT": xT,
                "wqT": np.ascontiguousarray(wq_c.T).astype(BF16),
                "wkvT": np.ascontiguousarray(wkv_c.T).astype(BF16),
                "woT": np.ascontiguousarray(woT_c).astype(BF16),
                "trimask": tri,
            }
        )
    return in_maps


def _run(in_maps, trace=False):
    import sys
    if "/opt/trn_rl_repo" not in sys.path:
        sys.path.insert(0, "/opt/trn_rl_repo")
    from concourse.bass_utils import run_bass_kernel_spmd

    nc = _get_nc()
    res = run_bass_kernel_spmd(nc, in_maps, list(range(NCORES)), trace=trace)
    return res


def kernel(x, wq, wk, wv, wo):
    in_maps = _prep_inputs(x, wq, wk, wv, wo)
    res = _run(in_maps)
    parts = np.stack([np.asarray(r["out"], np.float32) for r in res.results])
    out = parts.sum(axis=0, dtype=np.float64).astype(np.float32)
    return out.reshape(1, S, DIM)
